# revision 22
# baseline (speedup 1.0000x reference)
"""Trainium2 Bass kernel for nn_DeconvSlimCapsule3D (ConvTranspose3d capsule
layer with sabour dynamic routing), SPMD across 8 NeuronCores.

Sharding: core c = b*4 + s  (b = batch in {0,1}, s = D-slab in {0..3}).
Each core computes output D-planes [8s, 8s+8) of the 32^3 volume for batch b
from a 6-plane halo'd input slab. Zero inter-core communication.

v2 (optimized):
  - parities processed in PAIRS: the [64, pos] routing-chain tensors of two
    parities are packed into 128 partitions (even parity rows 0-63, odd
    64-127), halving ScalarE op count for the softmax/norm chain.
  - row map within a parity: r(j, od) = 32*(od//4) + 4*j + (od%4), so the
    oa-reduction splits into two K=64 matmuls per j (caps 0-63 -> strip
    [0,32), caps 64-127 -> strip [32,64)) that run CONCURRENTLY on disjoint
    PE row/col groups (tile_position inferred from base partitions).
  - iter-0 preact comes from a pre-summed x (xsum = sum_j x_j, one DVE tree
    over the input slab) through the same deconv stationary: kills the
    iter-0 reduction trees entirely.
  - j-sums (preact = sum_j votes_j * route_j) run on the PE as accumulated
    identity matmuls (two concurrent K=64 tiles), not DVE trees.
  - squares (votes^2, preact^2) run on the otherwise-idle GpSimd engine
    (split with DVE by a tunable constant).
  - route expansion od->caps stays on the PE (K=64, even/odd parity tiles
    concurrent); prods muls read the expansion from PSUM.
"""
import numpy as np
import ml_dtypes

B, IN_DIM, OUT_DIM, IN_ATOMS, OUT_ATOMS = 2, 8, 8, 16, 16
K, STRIDE, PAD = 4, 2, 1
CH = IN_ATOMS
D = 16
DO = 32
SLAB = 6 * 18 * 18  # 1944
BF16 = ml_dtypes.bfloat16

_CACHE = {}

# engine-split tuning: j < SQ_DVE_JS squares on DVE, rest on GpSimd
SQ_DVE_JS = 8


def _rowmap(j, od):
    return 32 * (od // 4) + 4 * j + (od % 4)


# ---------------- host-side prep ----------------

def _k_tap(r, d):
    return 3 - 2 * d if r == 0 else 2 - 2 * d


def _build_wcol(w):
    wcol = np.zeros((8, 128, 128), np.float32)
    for p in range(8):
        rd, rh, rw = p >> 2 & 1, p >> 1 & 1, p & 1
        for t in range(8):
            dd, dh, dw = t >> 2 & 1, t >> 1 & 1, t & 1
            kk = (_k_tap(rd, dd), _k_tap(rh, dh), _k_tap(rw, dw))
            wcol[p, t * 16:t * 16 + 16, :] = w[:, :, kk[0], kk[1], kk[2]]
    return wcol


def _make_xrep(x, b, s):
    slab = np.zeros((IN_DIM, CH, 6, 18, 18), np.float32)
    for j0 in range(6):
        i = 4 * s - 1 + j0
        if 0 <= i < D:
            slab[:, :, j0, 1:17, 1:17] = x[b, :, :, i]
    flat = slab.reshape(IN_DIM, CH, SLAB)
    xrep = np.zeros((128, IN_DIM * SLAB), np.float32)
    for t in range(8):
        dd, dh, dw = t >> 2 & 1, t >> 1 & 1, t & 1
        off = dd * 324 + dh * 18 + dw
        n = SLAB - off
        for j in range(IN_DIM):
            xrep[t * 16:t * 16 + 16, j * SLAB:j * SLAB + n] = flat[j, :, off:]
    return xrep


def _host_constants(w, deconv_b, routing_bias):
    wcol = _build_wcol(w)                                   # [8,128,128]
    wcol_d = wcol.transpose(1, 0, 2).reshape(128, 8 * 128)  # [K, (p, M)]
    # oa-reduce, K=64-split: cap = od*16+oa (half h = od//4); out strip row
    # m = 4j + od%4 within [32h, 32h+32) (+64 for odd parity via out AP).
    ored = np.zeros((128, 8 * 64), np.float32)
    for j in range(8):
        for od in range(8):
            for oa in range(16):
                ored[od * 16 + oa, 64 * j + _rowmap(j, od)] = 1.0
    # caps -> od sum (K=128): even parity -> cols 0-7, odd -> cols 8-15
    o1e = np.zeros((128, 16), np.float32)
    o1o = np.zeros((128, 16), np.float32)
    for od in range(8):
        o1e[od * 16:(od + 1) * 16, od] = 1.0
        o1o[od * 16:(od + 1) * 16, 8 + od] = 1.0
    # softmax Z: sum over od within each j (pair-packed rows)
    osm2 = np.zeros((128, 16), np.float32)
    for j in range(8):
        for od in range(8):
            osm2[_rowmap(j, od), j] = 1.0
            osm2[64 + _rowmap(j, od), 8 + j] = 1.0
    # lnz j -> (j,od) rows
    ez2 = np.zeros((16, 128), np.float32)
    for j in range(8):
        for od in range(8):
            ez2[j, _rowmap(j, od)] = 1.0
            ez2[8 + j, 64 + _rowmap(j, od)] = 1.0
    # rsq1 od -> (j,od) rows
    e2p = np.zeros((16, 128), np.float32)
    for od in range(8):
        for j in range(8):
            e2p[od, _rowmap(j, od)] = 1.0
            e2p[8 + od, 64 + _rowmap(j, od)] = 1.0
    # route rows -> caps, per j block (K=64 within parity half)
    ebig2 = np.zeros((128, 8 * 128), np.float32)
    for j in range(8):
        for od in range(8):
            for oa in range(16):
                ebig2[_rowmap(j, od), j * 128 + od * 16 + oa] = 1.0
                ebig2[64 + _rowmap(j, od), j * 128 + od * 16 + oa] = 1.0
    # identity for j-sum, both K=64 halves
    ident2 = np.eye(128, dtype=np.float32)
    # squash factor od -> caps (even f8 rows 0-7, odd rows 8-15)
    fbig_e = np.zeros((16, 128), np.float32)
    fbig_o = np.zeros((16, 128), np.float32)
    for od in range(8):
        fbig_e[od, od * 16:(od + 1) * 16] = 1.0
        fbig_o[8 + od, od * 16:(od + 1) * 16] = 1.0
    rbf = np.broadcast_to(routing_bias.reshape(-1), (128,)).astype(np.float32)
    bias3 = np.stack([deconv_b.astype(np.float32), rbf,
                      deconv_b.astype(np.float32) + rbf], axis=1)
    return {
        "wcol": wcol_d.astype(BF16),
        "ored": ored.astype(BF16),
        "o1e": o1e.astype(BF16),
        "o1o": o1o.astype(BF16),
        "osm2": osm2.astype(BF16),
        "ez2": ez2.astype(np.float32),
        "e2p": e2p.astype(BF16),
        "ebig2": ebig2.astype(BF16),
        "ident2": ident2.astype(BF16),
        "fbig_e": fbig_e.astype(np.float32),
        "fbig_o": fbig_o.astype(np.float32),
        "bias3": bias3,
    }


# ---------------- bass kernel ----------------

def _build_nc():
    import concourse.bass as bass
    import concourse.tile as tile
    from concourse import bacc, mybir
    from contextlib import ExitStack

    f32 = mybir.dt.float32
    f32r = mybir.dt.float32r
    bf16 = mybir.dt.bfloat16
    AF = mybir.ActivationFunctionType
    ALU = mybir.AluOpType

    nc = bacc.Bacc("TRN2", target_bir_lowering=False, debug=False)

    xrep_d = nc.dram_tensor("xrep", [128, IN_DIM * SLAB], bf16, kind="ExternalInput").ap()
    wcol_d = nc.dram_tensor("wcol", [128, 8 * 128], bf16, kind="ExternalInput").ap()
    ored_d = nc.dram_tensor("ored", [128, 8 * 64], bf16, kind="ExternalInput").ap()
    o1e_d = nc.dram_tensor("o1e", [128, 16], bf16, kind="ExternalInput").ap()
    o1o_d = nc.dram_tensor("o1o", [128, 16], bf16, kind="ExternalInput").ap()
    osm2_d = nc.dram_tensor("osm2", [128, 16], bf16, kind="ExternalInput").ap()
    ez2_d = nc.dram_tensor("ez2", [16, 128], f32r, kind="ExternalInput").ap()
    e2p_d = nc.dram_tensor("e2p", [16, 128], bf16, kind="ExternalInput").ap()
    ebig2_d = nc.dram_tensor("ebig2", [128, 8 * 128], bf16, kind="ExternalInput").ap()
    ident2_d = nc.dram_tensor("ident2", [128, 128], bf16, kind="ExternalInput").ap()
    fbig_e_d = nc.dram_tensor("fbig_e", [16, 128], f32r, kind="ExternalInput").ap()
    fbig_o_d = nc.dram_tensor("fbig_o", [16, 128], f32r, kind="ExternalInput").ap()
    bias3_d = nc.dram_tensor("bias3", [128, 3], f32, kind="ExternalInput").ap()
    out_d = nc.dram_tensor("out", [128, 8 * 1024], f32, kind="ExternalOutput").ap()

    with tile.TileContext(nc) as tc, ExitStack() as ctx:
        consts = ctx.enter_context(tc.tile_pool(name="consts", bufs=1))
        xpool = ctx.enter_context(tc.tile_pool(name="xrep", bufs=1))
        vpool = ctx.enter_context(tc.tile_pool(name="votes", bufs=2))
        sqp = ctx.enter_context(tc.tile_pool(name="sq", bufs=2))
        prp = ctx.enter_context(tc.tile_pool(name="pr", bufs=2))
        prodp = ctx.enter_context(tc.tile_pool(name="prods", bufs=2))
        pap = ctx.enter_context(tc.tile_pool(name="pa", bufs=1))
        smallp = ctx.enter_context(tc.tile_pool(name="small", bufs=2))
        outp = ctx.enter_context(tc.tile_pool(name="out", bufs=1))
        ps5 = ctx.enter_context(tc.tile_pool(name="ps5", bufs=4, space="PSUM"))
        psR = ctx.enter_context(tc.tile_pool(name="psR", bufs=1, space="PSUM"))
        psS = ctx.enter_context(tc.tile_pool(name="psS", bufs=1, space="PSUM"))
        warmp = ctx.enter_context(tc.tile_pool(name="warm", bufs=1, space="PSUM"))

        xrep_sb = xpool.tile([128, IN_DIM * SLAB], bf16)
        nc.sync.dma_start(xrep_sb, xrep_d)
        wcol_sb = consts.tile([128, 8 * 128], bf16)
        nc.sync.dma_start(wcol_sb, wcol_d)
        ored_sb = consts.tile([128, 8 * 64], bf16)
        nc.sync.dma_start(ored_sb, ored_d)
        o1e_sb = consts.tile([128, 16], bf16)
        nc.sync.dma_start(o1e_sb, o1e_d)
        o1o_sb = consts.tile([128, 16], bf16)
        nc.sync.dma_start(o1o_sb, o1o_d)
        osm2_sb = consts.tile([128, 16], bf16)
        nc.sync.dma_start(osm2_sb, osm2_d)
        ez2_sb = consts.tile([16, 128], f32r)
        nc.sync.dma_start(ez2_sb, ez2_d)
        e2p_sb = consts.tile([16, 128], bf16)
        nc.sync.dma_start(e2p_sb, e2p_d)
        ebig2_sb = consts.tile([128, 8 * 128], bf16)
        nc.sync.dma_start(ebig2_sb, ebig2_d)
        ident2_sb = consts.tile([128, 128], bf16)
        nc.sync.dma_start(ident2_sb, ident2_d)
        fbig_e_sb = consts.tile([16, 128], f32r)
        nc.sync.dma_start(fbig_e_sb, fbig_e_d)
        fbig_o_sb = consts.tile([16, 128], f32r)
        nc.sync.dma_start(fbig_o_sb, fbig_o_d)
        bias_sb = consts.tile([128, 3], f32)
        nc.sync.dma_start(bias_sb, bias3_d)

        # xsum = sum_j xrep_j  (alternating accumulators, 7 DVE adds)
        xacc = [xpool.tile([128, SLAB], bf16, tag="xacc0", name="xacc0"),
                xpool.tile([128, SLAB], bf16, tag="xacc1", name="xacc1")]
        xa = xrep_sb[:, :].rearrange("p (j n) -> p j n", j=8)
        nc.vector.tensor_add(xacc[0], xa[:, 0, :], xa[:, 1, :])
        for j in range(2, 8):
            nc.vector.tensor_add(xacc[(j + 1) % 2], xacc[j % 2], xa[:, j, :])
        xsum_sb = xacc[0]

        def window_rhs(src, j, p, h):
            # moving operand: [128, (a' 2, bh 16, bw 16)] shifted window
            rd, rh, rw = p >> 2 & 1, p >> 1 & 1, p & 1
            base = (j * SLAB if src is xrep_sb else 0) \
                + rd * 324 + rh * 18 + rw + h * 648
            a = src[:, :]
            return bass.AP(tensor=a.tensor, offset=a.offset + base,
                           ap=[list(a.ap[0]), [324, 2], [18, 16], [1, 16]])

        def kw16(t, c=0):
            wps = warmp.tile([16, 64], f32, tag="w", name="warm")
            nc.tensor.matmul(wps, ident2_sb[:, 0:16], t[:, c:c + 64],
                             start=True, stop=True)

        def kwf(t, c=0):
            wps = warmp.tile([16, 64], f32, tag="w", name="warmf")
            nc.tensor.matmul(wps[0:3, :], bias_sb, t[:, c:c + 64],
                             start=True, stop=True)

        def phase_a(q):
            pair = (2 * q, 2 * q + 1)
            votes = [None, None]
            pa = [None, None]
            # ---- deconv votes + bias-evict; preact0 from xsum ----
            for par, p in enumerate(pair):
                votes[par] = vpool.tile([128, 8 * 1024], bf16, tag=f"votes{par}",
                                        name=f"votes{q}_{par}")
                for j in range(8):
                    for h in (0, 1):
                        vps = ps5.tile([128, 512], f32, tag="ps5")
                        nc.tensor.matmul(vps, wcol_sb[:, p * 128:(p + 1) * 128],
                                         window_rhs(xrep_sb, j, p, h),
                                         start=True, stop=True)
                        nc.scalar.activation(
                            votes[par][:, j * 1024 + h * 512:j * 1024 + h * 512 + 512],
                            vps, AF.Identity, bias=bias_sb[:, 0:1])
                pa[par] = pap.tile([128, 1024], bf16, tag=f"pa{par}", bufs=3,
                                   name=f"pa0_{q}_{par}")
                for h in (0, 1):
                    pps = ps5.tile([128, 512], f32, tag="ps5")
                    nc.tensor.matmul(pps, wcol_sb[:, p * 128:(p + 1) * 128],
                                     window_rhs(xsum_sb, 0, p, h),
                                     start=True, stop=True)
                    nc.scalar.activation(pa[par][:, h * 512:h * 512 + 512], pps,
                                         AF.Identity, scale=0.125,
                                         bias=bias_sb[:, 2:3])
            # ---- votes^2 -> n2 (pair-packed [128,1024]) -> rsq2 ----
            n2ps = [ps5.tile([128, 512], f32, tag="ps5", name=f"n2ps{nh}")
                    for nh in (0, 1)]
            for par in range(2):
                for j in range(8):
                    sq = sqp.tile([128, 1024], bf16, tag="sq")
                    nc.vector.tensor_mul(sq, votes[par][:, j * 1024:(j + 1) * 1024],
                                         votes[par][:, j * 1024:(j + 1) * 1024])
                    for nh in (0, 1):
                        nc.tensor.matmul(
                            n2ps[nh][64 * par:64 * par + 64, :],
                            ored_sb[:, 64 * j:64 * j + 64],
                            sq[:, nh * 512:nh * 512 + 512],
                            start=(j == 0), stop=(j == 7))
            rsq2 = smallp.tile([128, 1024], bf16, tag="rsq2", bufs=2)
            for nh in (0, 1):
                lnn2 = smallp.tile([128, 512], f32, tag="t1", bufs=1)
                nc.scalar.activation(lnn2, n2ps[nh], AF.Ln)
                nc.scalar.activation(rsq2[:, nh * 512:nh * 512 + 512], lnn2,
                                     AF.Exp, scale=-0.5)
                kw16(rsq2, nh * 512)
            return {"q": q, "pair": pair, "votes": votes, "pa": pa, "rsq2": rsq2}

        def phase_b(st):
            q, pair, votes, pa, rsq2 = (st["q"], st["pair"], st["votes"],
                                        st["pa"], st["rsq2"])
            logits = smallp.tile([128, 1024], f32, tag="logits", bufs=1)
            for it in (1, 2):
                # ---- pr = votes * preact ; dot-reduce (pair-packed) ----
                dps = [ps5.tile([128, 512], f32, tag="ps5", name=f"dps{nh}")
                       for nh in (0, 1)]
                for par in range(2):
                    for j in range(8):
                        pr = prp.tile([128, 1024], bf16, tag="pr")
                        nc.vector.tensor_mul(pr,
                                             votes[par][:, j * 1024:(j + 1) * 1024],
                                             pa[par])
                        for nh in (0, 1):
                            nc.tensor.matmul(
                                dps[nh][64 * par:64 * par + 64, :],
                                ored_sb[:, 64 * j:64 * j + 64],
                                pr[:, nh * 512:nh * 512 + 512],
                                start=(j == 0), stop=(j == 7))
                # preact^2, n1 reduce (both parities into [16,512])
                psq = [None, None]
                for par in range(2):
                    psq[par] = smallp.tile([128, 1024], bf16, tag=f"psq{par}",
                                           bufs=1, name=f"psq{par}")
                    nc.vector.tensor_mul(psq[par], pa[par], pa[par])
                    kw16(psq[par])
                rsq1 = smallp.tile([16, 1024], bf16, tag="rsq1", bufs=2)
                for nh in (0, 1):
                    n1ps = psS.tile([16, 512], f32, tag="sm")
                    nc.tensor.matmul(n1ps, o1e_sb, psq[0][:, nh * 512:nh * 512 + 512],
                                     start=True, stop=False)
                    nc.tensor.matmul(n1ps, o1o_sb, psq[1][:, nh * 512:nh * 512 + 512],
                                     start=False, stop=True)
                    lnn1 = smallp.tile([16, 512], f32, tag="t2", bufs=2)
                    nc.scalar.activation(lnn1, n1ps, AF.Ln)
                    nc.scalar.activation(rsq1[:, nh * 512:nh * 512 + 512], lnn1,
                                         AF.Exp, scale=-0.5)
                    r1e = ps5.tile([128, 512], f32, tag="ps5")
                    nc.tensor.matmul(r1e, e2p_sb, rsq1[:, nh * 512:nh * 512 + 512],
                                     start=True, stop=True)
                    # cos = dps * rsq2 * rsq1e ; logits update
                    c1 = smallp.tile([128, 512], f32, tag="t3", bufs=1)
                    nc.vector.tensor_mul(c1, dps[nh], rsq2[:, nh * 512:nh * 512 + 512])
                    kwf(c1)
                    if it == 1:
                        nc.vector.tensor_mul(logits[:, nh * 512:nh * 512 + 512],
                                             c1, r1e)
                    else:
                        c2 = smallp.tile([128, 512], f32, tag="t4", bufs=1)
                        nc.vector.tensor_mul(c2, c1, r1e)
                        nc.vector.tensor_add(logits[:, nh * 512:nh * 512 + 512],
                                             logits[:, nh * 512:nh * 512 + 512], c2)
                    kwf(logits, nh * 512)
                # ---- softmax over od: route = exp(logits - lnZ_j) ----
                route = smallp.tile([128, 1024], bf16, tag="route", bufs=1)
                for nh in (0, 1):
                    el = smallp.tile([128, 512], bf16, tag="t5", bufs=1)
                    nc.scalar.activation(el, logits[:, nh * 512:nh * 512 + 512], AF.Exp)
                    kw16(el)
                    zps = psS.tile([16, 512], f32, tag="sm")
                    nc.tensor.matmul(zps, osm2_sb, el, start=True, stop=True)
                    lnz = smallp.tile([16, 512], f32r, tag="t6", bufs=1)
                    nc.scalar.activation(lnz, zps, AF.Ln)
                    lze = ps5.tile([128, 512], f32, tag="ps5")
                    nc.tensor.matmul(lze, ez2_sb, lnz, start=True, stop=True)
                    lm = smallp.tile([128, 512], f32, tag="t7", bufs=1)
                    nc.vector.tensor_sub(lm, logits[:, nh * 512:nh * 512 + 512], lze)
                    kwf(lm)
                    nc.scalar.activation(route[:, nh * 512:nh * 512 + 512], lm, AF.Exp)
                    kw16(route, nh * 512)
                # ---- preact = sum_j votes * route_exp + rb (PE j-sum) ----
                for par in range(2):
                    prods = [None] * 8
                    for j in range(8):
                        reps = psR.tile([128, 1024], f32, tag="reps")
                        for nh in (0, 1):
                            nc.tensor.matmul(
                                reps[:, nh * 512:nh * 512 + 512],
                                ebig2_sb[64 * par:64 * par + 64, j * 128:(j + 1) * 128],
                                route[64 * par:64 * par + 64, nh * 512:nh * 512 + 512],
                                start=True, stop=True)
                        prods[j] = prodp.tile([128, 1024], bf16, tag="prods",
                                              name=f"prods{j}")
                        nc.vector.tensor_mul(prods[j],
                                             votes[par][:, j * 1024:(j + 1) * 1024],
                                             reps)
                    pa[par] = pap.tile([128, 1024], bf16, tag=f"pa{par}", bufs=3,
                                       name=f"pan{it}_{q}_{par}")
                    for nh in (0, 1):
                        pps = ps5.tile([128, 512], f32, tag="ps5")
                        for j in range(8):
                            nc.tensor.matmul(
                                pps, ident2_sb,
                                prods[j][:, nh * 512:nh * 512 + 512],
                                start=(j == 0), stop=(j == 7))
                        nc.scalar.activation(pa[par][:, nh * 512:nh * 512 + 512], pps,
                                             AF.Identity, bias=bias_sb[:, 1:2])
                        kw16(pa[par], nh * 512)
            # ---- squash: out = preact * exp(0.5 ln(nsq) - ln(1+nsq)) ----
            psq2 = [None, None]
            for par in range(2):
                psq2[par] = smallp.tile([128, 1024], bf16, tag=f"psq{par}",
                                        bufs=1, name=f"psqs{par}")
                nc.vector.tensor_mul(psq2[par], pa[par], pa[par])
                kw16(psq2[par])
            for nh in (0, 1):
                yps = psS.tile([16, 512], f32, tag="sm")
                nc.tensor.matmul(yps, o1e_sb, psq2[0][:, nh * 512:nh * 512 + 512],
                                 start=True, stop=False)
                nc.tensor.matmul(yps, o1o_sb, psq2[1][:, nh * 512:nh * 512 + 512],
                                 start=False, stop=True)
                u1 = smallp.tile([16, 512], f32, tag="t8", bufs=2)
                nc.scalar.activation(u1, yps, AF.Ln)
                u2 = smallp.tile([16, 512], f32, tag="t9", bufs=2)
                nc.scalar.activation(u2, yps, AF.Ln, bias=1.0)
                farg = smallp.tile([16, 512], f32, tag="t10", bufs=2)
                nc.vector.scalar_tensor_tensor(farg, u1, 0.5, u2,
                                               op0=ALU.mult, op1=ALU.subtract)
                f8 = smallp.tile([16, 512], f32r, tag="t11", bufs=2)
                nc.scalar.activation(f8, farg, AF.Exp)
                for par, p in enumerate(pair):
                    fps = ps5.tile([128, 512], f32, tag="ps5")
                    nc.tensor.matmul(fps, fbig_e_sb if par == 0 else fbig_o_sb,
                                     f8, start=True, stop=True)
                    outt = outp.tile([128, 512], f32, tag="out")
                    nc.vector.tensor_mul(outt, pa[par][:, nh * 512:nh * 512 + 512], fps)
                    nc.sync.dma_start(
                        out_d[:, p * 1024 + nh * 512:p * 1024 + nh * 512 + 512], outt)

        for q in range(4):
            phase_b(phase_a(q))

    nc.compile()
    return nc


# ---------------- public entry point ----------------

def kernel(x, w, deconv_b, routing_bias):
    from concourse.bass_utils import run_bass_kernel_spmd

    x = np.asarray(x, np.float32)
    w = np.asarray(w, np.float32)
    deconv_b = np.asarray(deconv_b, np.float32)
    routing_bias = np.asarray(routing_bias, np.float32)

    if "nc" not in _CACHE:
        _CACHE["nc"] = _build_nc()
    nc = _CACHE["nc"]

    consts = _host_constants(w, deconv_b, routing_bias)
    in_maps = []
    for c in range(8):
        b, s = c // 4, c % 4
        m = dict(consts)
        m["xrep"] = _make_xrep(x, b, s).astype(BF16)
        in_maps.append(m)

    res = run_bass_kernel_spmd(nc, in_maps, list(range(8)),
                               trace=bool(_CACHE.get("trace")))
    _CACHE["last_res"] = res

    out = np.zeros((B, OUT_DIM, OUT_ATOMS, DO, DO, DO), np.float32)
    for c in range(8):
        b, s = c // 4, c % 4
        blk = np.asarray(res.results[c]["out"], np.float32)
        blk = blk.reshape(OUT_DIM, OUT_ATOMS, 2, 2, 2, 4, 16, 16)
        t = blk.transpose(0, 1, 5, 2, 6, 3, 7, 4)  # od,oa,a',rd,bh,rh,bw,rw
        out[b, :, :, 8 * s:8 * s + 8, :, :] = t.reshape(OUT_DIM, OUT_ATOMS, 8, 32, 32)
    return out


# revision 23
# speedup vs baseline: 1.0394x; 1.0394x over previous
"""Trainium2 Bass kernel for nn_DeconvSlimCapsule3D (ConvTranspose3d capsule
layer with sabour dynamic routing), SPMD across 8 NeuronCores.

Sharding: core c = b*4 + s  (b = batch in {0,1}, s = D-slab in {0..3}).
Each core computes output D-planes [8s, 8s+8) of the 32^3 volume for batch b
from a 6-plane halo'd input slab. Zero inter-core communication.

v2 (optimized):
  - parities processed in PAIRS: the [64, pos] routing-chain tensors of two
    parities are packed into 128 partitions (even parity rows 0-63, odd
    64-127), halving ScalarE op count for the softmax/norm chain.
  - row map within a parity: r(j, od) = 32*(od//4) + 4*j + (od%4), so the
    oa-reduction splits into two K=64 matmuls per j (caps 0-63 -> strip
    [0,32), caps 64-127 -> strip [32,64)) that run CONCURRENTLY on disjoint
    PE row/col groups (tile_position inferred from base partitions).
  - iter-0 preact comes from a pre-summed x (xsum = sum_j x_j, one DVE tree
    over the input slab) through the same deconv stationary: kills the
    iter-0 reduction trees entirely.
  - j-sums (preact = sum_j votes_j * route_j) run on the PE as accumulated
    identity matmuls (two concurrent K=64 tiles), not DVE trees.
  - squares (votes^2, preact^2) run on the otherwise-idle GpSimd engine
    (split with DVE by a tunable constant).
  - route expansion od->caps stays on the PE (K=64, even/odd parity tiles
    concurrent); prods muls read the expansion from PSUM.
"""
import numpy as np
import ml_dtypes

B, IN_DIM, OUT_DIM, IN_ATOMS, OUT_ATOMS = 2, 8, 8, 16, 16
K, STRIDE, PAD = 4, 2, 1
CH = IN_ATOMS
D = 16
DO = 32
SLAB = 6 * 18 * 18  # 1944
BF16 = ml_dtypes.bfloat16

_CACHE = {}

# engine-split tuning: j < SQ_DVE_JS squares on DVE, rest on GpSimd
SQ_DVE_JS = 8


def _rowmap(j, od):
    return 32 * (od // 4) + 4 * j + (od % 4)


# ---------------- host-side prep ----------------

def _k_tap(r, d):
    return 3 - 2 * d if r == 0 else 2 - 2 * d


def _build_wcol(w):
    wcol = np.zeros((8, 128, 128), np.float32)
    for p in range(8):
        rd, rh, rw = p >> 2 & 1, p >> 1 & 1, p & 1
        for t in range(8):
            dd, dh, dw = t >> 2 & 1, t >> 1 & 1, t & 1
            kk = (_k_tap(rd, dd), _k_tap(rh, dh), _k_tap(rw, dw))
            wcol[p, t * 16:t * 16 + 16, :] = w[:, :, kk[0], kk[1], kk[2]]
    return wcol


def _make_xrep(x, b, s):
    slab = np.zeros((IN_DIM, CH, 6, 18, 18), np.float32)
    for j0 in range(6):
        i = 4 * s - 1 + j0
        if 0 <= i < D:
            slab[:, :, j0, 1:17, 1:17] = x[b, :, :, i]
    flat = slab.reshape(IN_DIM, CH, SLAB)
    xrep = np.zeros((128, IN_DIM * SLAB), np.float32)
    for t in range(8):
        dd, dh, dw = t >> 2 & 1, t >> 1 & 1, t & 1
        off = dd * 324 + dh * 18 + dw
        n = SLAB - off
        for j in range(IN_DIM):
            xrep[t * 16:t * 16 + 16, j * SLAB:j * SLAB + n] = flat[j, :, off:]
    return xrep


def _host_constants(w, deconv_b, routing_bias):
    wcol = _build_wcol(w)                                   # [8,128,128]
    wcol_d = wcol.transpose(1, 0, 2).reshape(128, 8 * 128)  # [K, (p, M)]
    # oa-reduce, K=64-split: cap = od*16+oa (half h = od//4); out strip row
    # m = 4j + od%4 within [32h, 32h+32) (+64 for odd parity via out AP).
    ored = np.zeros((128, 8 * 64), np.float32)
    for j in range(8):
        for od in range(8):
            for oa in range(16):
                ored[od * 16 + oa, 64 * j + _rowmap(j, od)] = 1.0
    # caps -> od sum (K=128): even parity -> cols 0-7, odd -> cols 8-15
    o1e = np.zeros((128, 16), np.float32)
    o1o = np.zeros((128, 16), np.float32)
    for od in range(8):
        o1e[od * 16:(od + 1) * 16, od] = 1.0
        o1o[od * 16:(od + 1) * 16, 8 + od] = 1.0
    # softmax Z: sum over od within each j (pair-packed rows)
    osm2 = np.zeros((128, 16), np.float32)
    for j in range(8):
        for od in range(8):
            osm2[_rowmap(j, od), j] = 1.0
            osm2[64 + _rowmap(j, od), 8 + j] = 1.0
    # lnz j -> (j,od) rows
    ez2 = np.zeros((16, 128), np.float32)
    for j in range(8):
        for od in range(8):
            ez2[j, _rowmap(j, od)] = 1.0
            ez2[8 + j, 64 + _rowmap(j, od)] = 1.0
    # rsq1 od -> (j,od) rows
    e2p = np.zeros((16, 128), np.float32)
    for od in range(8):
        for j in range(8):
            e2p[od, _rowmap(j, od)] = 1.0
            e2p[8 + od, 64 + _rowmap(j, od)] = 1.0
    # route rows -> caps, per j block (K=64 within parity half)
    ebig2 = np.zeros((128, 8 * 128), np.float32)
    for j in range(8):
        for od in range(8):
            for oa in range(16):
                ebig2[_rowmap(j, od), j * 128 + od * 16 + oa] = 1.0
                ebig2[64 + _rowmap(j, od), j * 128 + od * 16 + oa] = 1.0
    # identity for j-sum, both K=64 halves
    ident2 = np.eye(128, dtype=np.float32)
    # squash factor od -> caps (even f8 rows 0-7, odd rows 8-15)
    fbig_e = np.zeros((16, 128), np.float32)
    fbig_o = np.zeros((16, 128), np.float32)
    for od in range(8):
        fbig_e[od, od * 16:(od + 1) * 16] = 1.0
        fbig_o[8 + od, od * 16:(od + 1) * 16] = 1.0
    rbf = np.broadcast_to(routing_bias.reshape(-1), (128,)).astype(np.float32)
    bias3 = np.stack([deconv_b.astype(np.float32), rbf,
                      deconv_b.astype(np.float32) + rbf], axis=1)
    return {
        "wcol": wcol_d.astype(BF16),
        "ored": ored.astype(BF16),
        "o1e": o1e.astype(BF16),
        "o1o": o1o.astype(BF16),
        "osm2": osm2.astype(BF16),
        "ez2": ez2.astype(np.float32),
        "e2p": e2p.astype(BF16),
        "ebig2": ebig2.astype(BF16),
        "ident2": ident2.astype(BF16),
        "fbig_e": fbig_e.astype(np.float32),
        "fbig_o": fbig_o.astype(np.float32),
        "bias3": bias3,
    }


# ---------------- bass kernel ----------------

def _build_nc():
    import concourse.bass as bass
    import concourse.tile as tile
    from concourse import bacc, mybir
    from contextlib import ExitStack

    f32 = mybir.dt.float32
    f32r = mybir.dt.float32r
    bf16 = mybir.dt.bfloat16
    AF = mybir.ActivationFunctionType
    ALU = mybir.AluOpType

    nc = bacc.Bacc("TRN2", target_bir_lowering=False, debug=False)

    xrep_d = nc.dram_tensor("xrep", [128, IN_DIM * SLAB], bf16, kind="ExternalInput").ap()
    wcol_d = nc.dram_tensor("wcol", [128, 8 * 128], bf16, kind="ExternalInput").ap()
    ored_d = nc.dram_tensor("ored", [128, 8 * 64], bf16, kind="ExternalInput").ap()
    o1e_d = nc.dram_tensor("o1e", [128, 16], bf16, kind="ExternalInput").ap()
    o1o_d = nc.dram_tensor("o1o", [128, 16], bf16, kind="ExternalInput").ap()
    osm2_d = nc.dram_tensor("osm2", [128, 16], bf16, kind="ExternalInput").ap()
    ez2_d = nc.dram_tensor("ez2", [16, 128], f32r, kind="ExternalInput").ap()
    e2p_d = nc.dram_tensor("e2p", [16, 128], bf16, kind="ExternalInput").ap()
    ebig2_d = nc.dram_tensor("ebig2", [128, 8 * 128], bf16, kind="ExternalInput").ap()
    ident2_d = nc.dram_tensor("ident2", [128, 128], bf16, kind="ExternalInput").ap()
    fbig_e_d = nc.dram_tensor("fbig_e", [16, 128], f32r, kind="ExternalInput").ap()
    fbig_o_d = nc.dram_tensor("fbig_o", [16, 128], f32r, kind="ExternalInput").ap()
    bias3_d = nc.dram_tensor("bias3", [128, 3], f32, kind="ExternalInput").ap()
    out_d = nc.dram_tensor("out", [128, 8 * 1024], f32, kind="ExternalOutput").ap()

    with tile.TileContext(nc) as tc, ExitStack() as ctx:
        consts = ctx.enter_context(tc.tile_pool(name="consts", bufs=1))
        xpool = ctx.enter_context(tc.tile_pool(name="xrep", bufs=1))
        vpool = ctx.enter_context(tc.tile_pool(name="votes", bufs=2))
        sqp = ctx.enter_context(tc.tile_pool(name="sq", bufs=2))
        prp = ctx.enter_context(tc.tile_pool(name="pr", bufs=2))
        prodp = ctx.enter_context(tc.tile_pool(name="prods", bufs=2))
        pap = ctx.enter_context(tc.tile_pool(name="pa", bufs=1))
        smallp = ctx.enter_context(tc.tile_pool(name="small", bufs=2))
        outp = ctx.enter_context(tc.tile_pool(name="out", bufs=1))
        ps5 = ctx.enter_context(tc.tile_pool(name="ps5", bufs=4, space="PSUM"))
        psR = ctx.enter_context(tc.tile_pool(name="psR", bufs=1, space="PSUM"))
        psS = ctx.enter_context(tc.tile_pool(name="psS", bufs=2, space="PSUM"))

        xrep_sb = xpool.tile([128, IN_DIM * SLAB], bf16)
        nc.sync.dma_start(xrep_sb, xrep_d)
        wcol_sb = consts.tile([128, 8 * 128], bf16)
        nc.sync.dma_start(wcol_sb, wcol_d)
        ored_sb = consts.tile([128, 8 * 64], bf16)
        nc.sync.dma_start(ored_sb, ored_d)
        o1e_sb = consts.tile([128, 16], bf16)
        nc.sync.dma_start(o1e_sb, o1e_d)
        o1o_sb = consts.tile([128, 16], bf16)
        nc.sync.dma_start(o1o_sb, o1o_d)
        osm2_sb = consts.tile([128, 16], bf16)
        nc.sync.dma_start(osm2_sb, osm2_d)
        ez2_sb = consts.tile([16, 128], f32r)
        nc.sync.dma_start(ez2_sb, ez2_d)
        e2p_sb = consts.tile([16, 128], bf16)
        nc.sync.dma_start(e2p_sb, e2p_d)
        ebig2_sb = consts.tile([128, 8 * 128], bf16)
        nc.sync.dma_start(ebig2_sb, ebig2_d)
        ident2_sb = consts.tile([128, 128], bf16)
        nc.sync.dma_start(ident2_sb, ident2_d)
        fbig_e_sb = consts.tile([16, 128], f32r)
        nc.sync.dma_start(fbig_e_sb, fbig_e_d)
        fbig_o_sb = consts.tile([16, 128], f32r)
        nc.sync.dma_start(fbig_o_sb, fbig_o_d)
        bias_sb = consts.tile([128, 3], f32)
        nc.sync.dma_start(bias_sb, bias3_d)

        # xsum = sum_j xrep_j  (alternating accumulators, 7 DVE adds)
        xacc = [xpool.tile([128, SLAB], bf16, tag="xacc0", name="xacc0"),
                xpool.tile([128, SLAB], bf16, tag="xacc1", name="xacc1")]
        xa = xrep_sb[:, :].rearrange("p (j n) -> p j n", j=8)
        nc.vector.tensor_add(xacc[0], xa[:, 0, :], xa[:, 1, :])
        for j in range(2, 8):
            nc.vector.tensor_add(xacc[(j + 1) % 2], xacc[j % 2], xa[:, j, :])
        xsum_sb = xacc[0]

        def window_rhs(src, j, p, h):
            # moving operand: [128, (a' 2, bh 16, bw 16)] shifted window
            rd, rh, rw = p >> 2 & 1, p >> 1 & 1, p & 1
            base = (j * SLAB if src is xrep_sb else 0) \
                + rd * 324 + rh * 18 + rw + h * 648
            a = src[:, :]
            return bass.AP(tensor=a.tensor, offset=a.offset + base,
                           ap=[list(a.ap[0]), [324, 2], [18, 16], [1, 16]])

        def phase_a(q):
            pair = (2 * q, 2 * q + 1)
            votes = [None, None]
            pa = [None, None]
            # ---- deconv votes + bias-evict; preact0 from xsum ----
            for par, p in enumerate(pair):
                votes[par] = vpool.tile([128, 8 * 1024], bf16, tag=f"votes{par}",
                                        name=f"votes{q}_{par}")
                for j in range(8):
                    for h in (0, 1):
                        vps = ps5.tile([128, 512], f32, tag="ps5")
                        nc.tensor.matmul(vps, wcol_sb[:, p * 128:(p + 1) * 128],
                                         window_rhs(xrep_sb, j, p, h),
                                         start=True, stop=True)
                        nc.scalar.activation(
                            votes[par][:, j * 1024 + h * 512:j * 1024 + h * 512 + 512],
                            vps, AF.Identity, bias=bias_sb[:, 0:1])
                pa[par] = pap.tile([128, 1024], bf16, tag=f"pa{par}", bufs=3,
                                   name=f"pa0_{q}_{par}")
                for h in (0, 1):
                    pps = ps5.tile([128, 512], f32, tag="ps5")
                    nc.tensor.matmul(pps, wcol_sb[:, p * 128:(p + 1) * 128],
                                     window_rhs(xsum_sb, 0, p, h),
                                     start=True, stop=True)
                    nc.scalar.activation(pa[par][:, h * 512:h * 512 + 512], pps,
                                         AF.Identity, scale=0.125,
                                         bias=bias_sb[:, 2:3])
            # ---- votes^2 -> n2 (pair-packed [128,1024]) -> rsq2 ----
            n2ps = [ps5.tile([128, 512], f32, tag="ps5", name=f"n2ps{nh}")
                    for nh in (0, 1)]
            for par in range(2):
                for j in range(8):
                    sq = sqp.tile([128, 1024], bf16, tag="sq")
                    nc.vector.tensor_mul(sq, votes[par][:, j * 1024:(j + 1) * 1024],
                                         votes[par][:, j * 1024:(j + 1) * 1024])
                    for nh in (0, 1):
                        nc.tensor.matmul(
                            n2ps[nh][64 * par:64 * par + 64, :],
                            ored_sb[:, 64 * j:64 * j + 64],
                            sq[:, nh * 512:nh * 512 + 512],
                            start=(j == 0), stop=(j == 7))
            rsq2 = smallp.tile([128, 1024], bf16, tag="rsq2", bufs=2)
            for nh in (0, 1):
                lnn2 = smallp.tile([128, 512], f32, tag="t1", bufs=1)
                nc.scalar.activation(lnn2, n2ps[nh], AF.Ln)
                nc.scalar.activation(rsq2[:, nh * 512:nh * 512 + 512], lnn2,
                                     AF.Exp, scale=-0.5)
            return {"q": q, "pair": pair, "votes": votes, "pa": pa, "rsq2": rsq2}

        def phase_b(st):
            q, pair, votes, pa, rsq2 = (st["q"], st["pair"], st["votes"],
                                        st["pa"], st["rsq2"])
            logits = smallp.tile([128, 1024], f32, tag="logits", bufs=1)
            for it in (1, 2):
                # ---- pr = votes * preact ; dot-reduce (pair-packed) ----
                dps = [ps5.tile([128, 512], f32, tag="ps5", name=f"dps{nh}")
                       for nh in (0, 1)]
                for par in range(2):
                    for j in range(8):
                        pr = prp.tile([128, 1024], bf16, tag="pr")
                        nc.vector.tensor_mul(pr,
                                             votes[par][:, j * 1024:(j + 1) * 1024],
                                             pa[par])
                        for nh in (0, 1):
                            nc.tensor.matmul(
                                dps[nh][64 * par:64 * par + 64, :],
                                ored_sb[:, 64 * j:64 * j + 64],
                                pr[:, nh * 512:nh * 512 + 512],
                                start=(j == 0), stop=(j == 7))
                # preact^2, n1 reduce (both parities into [16,512])
                psq = [None, None]
                for par in range(2):
                    psq[par] = smallp.tile([128, 1024], bf16, tag=f"psq{par}",
                                           bufs=1, name=f"psq{par}")
                    nc.vector.tensor_mul(psq[par], pa[par], pa[par])
                rsq1 = smallp.tile([16, 1024], bf16, tag="rsq1", bufs=2)
                for nh in (0, 1):
                    n1ps = psS.tile([16, 512], f32, tag="sm")
                    nc.tensor.matmul(n1ps, o1e_sb, psq[0][:, nh * 512:nh * 512 + 512],
                                     start=True, stop=False)
                    nc.tensor.matmul(n1ps, o1o_sb, psq[1][:, nh * 512:nh * 512 + 512],
                                     start=False, stop=True)
                    lnn1 = smallp.tile([16, 512], f32, tag="t2", bufs=2)
                    nc.scalar.activation(lnn1, n1ps, AF.Ln)
                    nc.scalar.activation(rsq1[:, nh * 512:nh * 512 + 512], lnn1,
                                         AF.Exp, scale=-0.5)
                    r1e = ps5.tile([128, 512], f32, tag="ps5")
                    nc.tensor.matmul(r1e, e2p_sb, rsq1[:, nh * 512:nh * 512 + 512],
                                     start=True, stop=True)
                    # cos = dps * rsq2 * rsq1e ; logits update
                    c1 = smallp.tile([128, 512], f32, tag="t3", bufs=1)
                    nc.vector.tensor_mul(c1, dps[nh], rsq2[:, nh * 512:nh * 512 + 512])
                    if it == 1:
                        nc.vector.tensor_mul(logits[:, nh * 512:nh * 512 + 512],
                                             c1, r1e)
                    else:
                        c2 = smallp.tile([128, 512], f32, tag="t4", bufs=1)
                        nc.vector.tensor_mul(c2, c1, r1e)
                        nc.vector.tensor_add(logits[:, nh * 512:nh * 512 + 512],
                                             logits[:, nh * 512:nh * 512 + 512], c2)
                # ---- softmax over od: route = exp(logits - lnZ_j) ----
                route = smallp.tile([128, 1024], bf16, tag="route", bufs=1)
                for nh in (0, 1):
                    el = smallp.tile([128, 512], bf16, tag="t5", bufs=1)
                    nc.scalar.activation(el, logits[:, nh * 512:nh * 512 + 512], AF.Exp)
                    zps = psS.tile([16, 512], f32, tag="sm")
                    nc.tensor.matmul(zps, osm2_sb, el, start=True, stop=True)
                    lnz = smallp.tile([16, 512], f32r, tag="t6", bufs=1)
                    nc.scalar.activation(lnz, zps, AF.Ln)
                    lze = ps5.tile([128, 512], f32, tag="ps5")
                    nc.tensor.matmul(lze, ez2_sb, lnz, start=True, stop=True)
                    lm = smallp.tile([128, 512], f32, tag="t7", bufs=1)
                    nc.vector.tensor_sub(lm, logits[:, nh * 512:nh * 512 + 512], lze)
                    nc.scalar.activation(route[:, nh * 512:nh * 512 + 512], lm, AF.Exp)
                # ---- preact = sum_j votes * route_exp + rb (PE j-sum) ----
                for par in range(2):
                    prods = [None] * 8
                    for j in range(8):
                        reps = psR.tile([128, 1024], f32, tag="reps")
                        for nh in (0, 1):
                            nc.tensor.matmul(
                                reps[:, nh * 512:nh * 512 + 512],
                                ebig2_sb[64 * par:64 * par + 64, j * 128:(j + 1) * 128],
                                route[64 * par:64 * par + 64, nh * 512:nh * 512 + 512],
                                start=True, stop=True)
                        prods[j] = prodp.tile([128, 1024], bf16, tag="prods",
                                              name=f"prods{j}")
                        nc.vector.tensor_mul(prods[j],
                                             votes[par][:, j * 1024:(j + 1) * 1024],
                                             reps)
                    pa[par] = pap.tile([128, 1024], bf16, tag=f"pa{par}", bufs=3,
                                       name=f"pan{it}_{q}_{par}")
                    for nh in (0, 1):
                        pps = ps5.tile([128, 512], f32, tag="ps5")
                        for j in range(8):
                            nc.tensor.matmul(
                                pps, ident2_sb,
                                prods[j][:, nh * 512:nh * 512 + 512],
                                start=(j == 0), stop=(j == 7))
                        nc.scalar.activation(pa[par][:, nh * 512:nh * 512 + 512], pps,
                                             AF.Identity, bias=bias_sb[:, 1:2])
            # ---- squash: out = preact * exp(0.5 ln(nsq) - ln(1+nsq)) ----
            psq2 = [None, None]
            for par in range(2):
                psq2[par] = smallp.tile([128, 1024], bf16, tag=f"psq{par}",
                                        bufs=1, name=f"psqs{par}")
                nc.vector.tensor_mul(psq2[par], pa[par], pa[par])
            for nh in (0, 1):
                yps = psS.tile([16, 512], f32, tag="sm")
                nc.tensor.matmul(yps, o1e_sb, psq2[0][:, nh * 512:nh * 512 + 512],
                                 start=True, stop=False)
                nc.tensor.matmul(yps, o1o_sb, psq2[1][:, nh * 512:nh * 512 + 512],
                                 start=False, stop=True)
                u1 = smallp.tile([16, 512], f32, tag="t8", bufs=2)
                nc.scalar.activation(u1, yps, AF.Ln)
                u2 = smallp.tile([16, 512], f32, tag="t9", bufs=2)
                nc.scalar.activation(u2, yps, AF.Ln, bias=1.0)
                farg = smallp.tile([16, 512], f32, tag="t10", bufs=2)
                nc.vector.scalar_tensor_tensor(farg, u1, 0.5, u2,
                                               op0=ALU.mult, op1=ALU.subtract)
                f8 = smallp.tile([16, 512], f32r, tag="t11", bufs=2)
                nc.scalar.activation(f8, farg, AF.Exp)
                for par, p in enumerate(pair):
                    fps = ps5.tile([128, 512], f32, tag="ps5")
                    nc.tensor.matmul(fps, fbig_e_sb if par == 0 else fbig_o_sb,
                                     f8, start=True, stop=True)
                    outt = outp.tile([128, 512], f32, tag="out")
                    nc.vector.tensor_mul(outt, pa[par][:, nh * 512:nh * 512 + 512], fps)
                    nc.sync.dma_start(
                        out_d[:, p * 1024 + nh * 512:p * 1024 + nh * 512 + 512], outt)

        for q in range(4):
            phase_b(phase_a(q))

    nc.compile()
    return nc


# ---------------- public entry point ----------------

def kernel(x, w, deconv_b, routing_bias):
    from concourse.bass_utils import run_bass_kernel_spmd

    x = np.asarray(x, np.float32)
    w = np.asarray(w, np.float32)
    deconv_b = np.asarray(deconv_b, np.float32)
    routing_bias = np.asarray(routing_bias, np.float32)

    if "nc" not in _CACHE:
        _CACHE["nc"] = _build_nc()
    nc = _CACHE["nc"]

    consts = _host_constants(w, deconv_b, routing_bias)
    in_maps = []
    for c in range(8):
        b, s = c // 4, c % 4
        m = dict(consts)
        m["xrep"] = _make_xrep(x, b, s).astype(BF16)
        in_maps.append(m)

    res = run_bass_kernel_spmd(nc, in_maps, list(range(8)),
                               trace=bool(_CACHE.get("trace")))
    _CACHE["last_res"] = res

    out = np.zeros((B, OUT_DIM, OUT_ATOMS, DO, DO, DO), np.float32)
    for c in range(8):
        b, s = c // 4, c % 4
        blk = np.asarray(res.results[c]["out"], np.float32)
        blk = blk.reshape(OUT_DIM, OUT_ATOMS, 2, 2, 2, 4, 16, 16)
        t = blk.transpose(0, 1, 5, 2, 6, 3, 7, 4)  # od,oa,a',rd,bh,rh,bw,rw
        out[b, :, :, 8 * s:8 * s + 8, :, :] = t.reshape(OUT_DIM, OUT_ATOMS, 8, 32, 32)
    return out


# revision 24
# speedup vs baseline: 1.0640x; 1.0237x over previous
"""Trainium2 Bass kernel for nn_DeconvSlimCapsule3D (ConvTranspose3d capsule
layer with sabour dynamic routing), SPMD across 8 NeuronCores.

Sharding: core c = b*4 + s  (b = batch in {0,1}, s = D-slab in {0..3}).
Each core computes output D-planes [8s, 8s+8) of the 32^3 volume for batch b
from a 6-plane halo'd input slab. Zero inter-core communication.

v2 (optimized):
  - parities processed in PAIRS: the [64, pos] routing-chain tensors of two
    parities are packed into 128 partitions (even parity rows 0-63, odd
    64-127), halving ScalarE op count for the softmax/norm chain.
  - row map within a parity: r(j, od) = 32*(od//4) + 4*j + (od%4), so the
    oa-reduction splits into two K=64 matmuls per j (caps 0-63 -> strip
    [0,32), caps 64-127 -> strip [32,64)) that run CONCURRENTLY on disjoint
    PE row/col groups (tile_position inferred from base partitions).
  - iter-0 preact comes from a pre-summed x (xsum = sum_j x_j, one DVE tree
    over the input slab) through the same deconv stationary: kills the
    iter-0 reduction trees entirely.
  - j-sums (preact = sum_j votes_j * route_j) run on the PE as accumulated
    identity matmuls (two concurrent K=64 tiles), not DVE trees.
  - squares (votes^2, preact^2) run on the otherwise-idle GpSimd engine
    (split with DVE by a tunable constant).
  - route expansion od->caps stays on the PE (K=64, even/odd parity tiles
    concurrent); prods muls read the expansion from PSUM.
"""
import numpy as np
import ml_dtypes

B, IN_DIM, OUT_DIM, IN_ATOMS, OUT_ATOMS = 2, 8, 8, 16, 16
K, STRIDE, PAD = 4, 2, 1
CH = IN_ATOMS
D = 16
DO = 32
SLAB = 6 * 18 * 18  # 1944
BF16 = ml_dtypes.bfloat16

_CACHE = {}

# engine-split tuning: j < SQ_DVE_JS squares on DVE, rest on GpSimd
SQ_DVE_JS = 8


def _rowmap(j, od):
    return 32 * (od // 4) + 4 * j + (od % 4)


# ---------------- host-side prep ----------------

def _k_tap(r, d):
    return 3 - 2 * d if r == 0 else 2 - 2 * d


def _build_wcol(w):
    wcol = np.zeros((8, 128, 128), np.float32)
    for p in range(8):
        rd, rh, rw = p >> 2 & 1, p >> 1 & 1, p & 1
        for t in range(8):
            dd, dh, dw = t >> 2 & 1, t >> 1 & 1, t & 1
            kk = (_k_tap(rd, dd), _k_tap(rh, dh), _k_tap(rw, dw))
            wcol[p, t * 16:t * 16 + 16, :] = w[:, :, kk[0], kk[1], kk[2]]
    return wcol


def _make_xrep(x, b, s):
    slab = np.zeros((IN_DIM, CH, 6, 18, 18), np.float32)
    for j0 in range(6):
        i = 4 * s - 1 + j0
        if 0 <= i < D:
            slab[:, :, j0, 1:17, 1:17] = x[b, :, :, i]
    flat = slab.reshape(IN_DIM, CH, SLAB)
    xrep = np.zeros((128, IN_DIM * SLAB), np.float32)
    for t in range(8):
        dd, dh, dw = t >> 2 & 1, t >> 1 & 1, t & 1
        off = dd * 324 + dh * 18 + dw
        n = SLAB - off
        for j in range(IN_DIM):
            xrep[t * 16:t * 16 + 16, j * SLAB:j * SLAB + n] = flat[j, :, off:]
    return xrep


def _host_constants(w, deconv_b, routing_bias):
    wcol = _build_wcol(w)                                   # [8,128,128]
    wcol_d = wcol.transpose(1, 0, 2).reshape(128, 8 * 128)  # [K, (p, M)]
    # oa-reduce, K=64-split: cap = od*16+oa (half h = od//4); out strip row
    # m = 4j + od%4 within [32h, 32h+32) (+64 for odd parity via out AP).
    ored = np.zeros((128, 8 * 64), np.float32)
    for j in range(8):
        for od in range(8):
            for oa in range(16):
                ored[od * 16 + oa, 64 * j + _rowmap(j, od)] = 1.0
    # caps -> od sum (K=128): even parity -> cols 0-7, odd -> cols 8-15
    o1e = np.zeros((128, 16), np.float32)
    o1o = np.zeros((128, 16), np.float32)
    for od in range(8):
        o1e[od * 16:(od + 1) * 16, od] = 1.0
        o1o[od * 16:(od + 1) * 16, 8 + od] = 1.0
    # softmax Z: sum over od within each j (pair-packed rows)
    osm2 = np.zeros((128, 16), np.float32)
    for j in range(8):
        for od in range(8):
            osm2[_rowmap(j, od), j] = 1.0
            osm2[64 + _rowmap(j, od), 8 + j] = 1.0
    # lnz j -> (j,od) rows
    ez2 = np.zeros((16, 128), np.float32)
    for j in range(8):
        for od in range(8):
            ez2[j, _rowmap(j, od)] = 1.0
            ez2[8 + j, 64 + _rowmap(j, od)] = 1.0
    # rsq1 od -> (j,od) rows
    e2p = np.zeros((16, 128), np.float32)
    for od in range(8):
        for j in range(8):
            e2p[od, _rowmap(j, od)] = 1.0
            e2p[8 + od, 64 + _rowmap(j, od)] = 1.0
    # route rows -> caps, per j block (K=64 within parity half)
    ebig2 = np.zeros((128, 8 * 128), np.float32)
    for j in range(8):
        for od in range(8):
            for oa in range(16):
                ebig2[_rowmap(j, od), j * 128 + od * 16 + oa] = 1.0
                ebig2[64 + _rowmap(j, od), j * 128 + od * 16 + oa] = 1.0
    # identity for j-sum, both K=64 halves
    ident2 = np.eye(128, dtype=np.float32)
    # squash factor od -> caps (even f8 rows 0-7, odd rows 8-15)
    fbig_e = np.zeros((16, 128), np.float32)
    fbig_o = np.zeros((16, 128), np.float32)
    for od in range(8):
        fbig_e[od, od * 16:(od + 1) * 16] = 1.0
        fbig_o[8 + od, od * 16:(od + 1) * 16] = 1.0
    rbf = np.broadcast_to(routing_bias.reshape(-1), (128,)).astype(np.float32)
    bias3 = np.stack([deconv_b.astype(np.float32), rbf,
                      deconv_b.astype(np.float32) + rbf], axis=1)
    return {
        "wcol": wcol_d.astype(BF16),
        "ored": ored.astype(BF16),
        "o1e": o1e.astype(BF16),
        "o1o": o1o.astype(BF16),
        "osm2": osm2.astype(BF16),
        "ez2": ez2.astype(np.float32),
        "e2p": e2p.astype(BF16),
        "ebig2": ebig2.astype(BF16),
        "ident2": ident2.astype(BF16),
        "fbig_e": fbig_e.astype(np.float32),
        "fbig_o": fbig_o.astype(np.float32),
        "bias3": bias3,
    }


# ---------------- bass kernel ----------------

def _build_nc():
    import concourse.bass as bass
    import concourse.tile as tile
    from concourse import bacc, mybir
    from contextlib import ExitStack

    f32 = mybir.dt.float32
    f32r = mybir.dt.float32r
    bf16 = mybir.dt.bfloat16
    AF = mybir.ActivationFunctionType
    ALU = mybir.AluOpType

    nc = bacc.Bacc("TRN2", target_bir_lowering=False, debug=False)

    xrep_d = nc.dram_tensor("xrep", [128, IN_DIM * SLAB], bf16, kind="ExternalInput").ap()
    wcol_d = nc.dram_tensor("wcol", [128, 8 * 128], bf16, kind="ExternalInput").ap()
    ored_d = nc.dram_tensor("ored", [128, 8 * 64], bf16, kind="ExternalInput").ap()
    o1e_d = nc.dram_tensor("o1e", [128, 16], bf16, kind="ExternalInput").ap()
    o1o_d = nc.dram_tensor("o1o", [128, 16], bf16, kind="ExternalInput").ap()
    osm2_d = nc.dram_tensor("osm2", [128, 16], bf16, kind="ExternalInput").ap()
    ez2_d = nc.dram_tensor("ez2", [16, 128], f32r, kind="ExternalInput").ap()
    e2p_d = nc.dram_tensor("e2p", [16, 128], bf16, kind="ExternalInput").ap()
    ebig2_d = nc.dram_tensor("ebig2", [128, 8 * 128], bf16, kind="ExternalInput").ap()
    ident2_d = nc.dram_tensor("ident2", [128, 128], bf16, kind="ExternalInput").ap()
    fbig_e_d = nc.dram_tensor("fbig_e", [16, 128], f32r, kind="ExternalInput").ap()
    fbig_o_d = nc.dram_tensor("fbig_o", [16, 128], f32r, kind="ExternalInput").ap()
    bias3_d = nc.dram_tensor("bias3", [128, 3], f32, kind="ExternalInput").ap()
    out_d = nc.dram_tensor("out", [128, 8 * 1024], f32, kind="ExternalOutput").ap()

    with tile.TileContext(nc) as tc, ExitStack() as ctx:
        consts = ctx.enter_context(tc.tile_pool(name="consts", bufs=1))
        xpool = ctx.enter_context(tc.tile_pool(name="xrep", bufs=1))
        vpool = ctx.enter_context(tc.tile_pool(name="votes", bufs=2))
        sqp = ctx.enter_context(tc.tile_pool(name="sq", bufs=2))
        prp = ctx.enter_context(tc.tile_pool(name="pr", bufs=3))
        prodp = ctx.enter_context(tc.tile_pool(name="prods", bufs=3))
        pap = ctx.enter_context(tc.tile_pool(name="pa", bufs=1))
        smallp = ctx.enter_context(tc.tile_pool(name="small", bufs=2))
        outp = ctx.enter_context(tc.tile_pool(name="out", bufs=2))
        ps5 = ctx.enter_context(tc.tile_pool(name="ps5", bufs=4, space="PSUM"))
        psR = ctx.enter_context(tc.tile_pool(name="psR", bufs=1, space="PSUM"))
        psS = ctx.enter_context(tc.tile_pool(name="psS", bufs=2, space="PSUM"))

        xrep_sb = xpool.tile([128, IN_DIM * SLAB], bf16)
        nc.sync.dma_start(xrep_sb, xrep_d)
        wcol_sb = consts.tile([128, 8 * 128], bf16)
        nc.sync.dma_start(wcol_sb, wcol_d)
        ored_sb = consts.tile([128, 8 * 64], bf16)
        nc.sync.dma_start(ored_sb, ored_d)
        o1e_sb = consts.tile([128, 16], bf16)
        nc.sync.dma_start(o1e_sb, o1e_d)
        o1o_sb = consts.tile([128, 16], bf16)
        nc.sync.dma_start(o1o_sb, o1o_d)
        osm2_sb = consts.tile([128, 16], bf16)
        nc.sync.dma_start(osm2_sb, osm2_d)
        ez2_sb = consts.tile([16, 128], f32r)
        nc.sync.dma_start(ez2_sb, ez2_d)
        e2p_sb = consts.tile([16, 128], bf16)
        nc.sync.dma_start(e2p_sb, e2p_d)
        ebig2_sb = consts.tile([128, 8 * 128], bf16)
        nc.sync.dma_start(ebig2_sb, ebig2_d)
        ident2_sb = consts.tile([128, 128], bf16)
        nc.sync.dma_start(ident2_sb, ident2_d)
        fbig_e_sb = consts.tile([16, 128], f32r)
        nc.sync.dma_start(fbig_e_sb, fbig_e_d)
        fbig_o_sb = consts.tile([16, 128], f32r)
        nc.sync.dma_start(fbig_o_sb, fbig_o_d)
        bias_sb = consts.tile([128, 3], f32)
        nc.sync.dma_start(bias_sb, bias3_d)

        # xsum = sum_j xrep_j  (alternating accumulators, 7 DVE adds)
        xacc = [xpool.tile([128, SLAB], bf16, tag="xacc0", name="xacc0"),
                xpool.tile([128, SLAB], bf16, tag="xacc1", name="xacc1")]
        xa = xrep_sb[:, :].rearrange("p (j n) -> p j n", j=8)
        nc.vector.tensor_add(xacc[0], xa[:, 0, :], xa[:, 1, :])
        for j in range(2, 8):
            nc.vector.tensor_add(xacc[(j + 1) % 2], xacc[j % 2], xa[:, j, :])
        xsum_sb = xacc[0]

        def window_rhs(src, j, p, h):
            # moving operand: [128, (a' 2, bh 16, bw 16)] shifted window
            rd, rh, rw = p >> 2 & 1, p >> 1 & 1, p & 1
            base = (j * SLAB if src is xrep_sb else 0) \
                + rd * 324 + rh * 18 + rw + h * 648
            a = src[:, :]
            return bass.AP(tensor=a.tensor, offset=a.offset + base,
                           ap=[list(a.ap[0]), [324, 2], [18, 16], [1, 16]])

        def phase_a(q):
            pair = (2 * q, 2 * q + 1)
            votes = [None, None]
            pa = [None, None]
            # ---- deconv votes + bias-evict; preact0 from xsum ----
            for par, p in enumerate(pair):
                votes[par] = vpool.tile([128, 8 * 1024], bf16, tag=f"votes{par}",
                                        name=f"votes{q}_{par}")
                for j in range(8):
                    for h in (0, 1):
                        vps = ps5.tile([128, 512], f32, tag="ps5")
                        nc.tensor.matmul(vps, wcol_sb[:, p * 128:(p + 1) * 128],
                                         window_rhs(xrep_sb, j, p, h),
                                         start=True, stop=True)
                        nc.scalar.activation(
                            votes[par][:, j * 1024 + h * 512:j * 1024 + h * 512 + 512],
                            vps, AF.Identity, bias=bias_sb[:, 0:1])
                pa[par] = pap.tile([128, 1024], bf16, tag=f"pa{par}", bufs=3,
                                   name=f"pa0_{q}_{par}")
                for h in (0, 1):
                    pps = ps5.tile([128, 512], f32, tag="ps5")
                    nc.tensor.matmul(pps, wcol_sb[:, p * 128:(p + 1) * 128],
                                     window_rhs(xsum_sb, 0, p, h),
                                     start=True, stop=True)
                    nc.scalar.activation(pa[par][:, h * 512:h * 512 + 512], pps,
                                         AF.Identity, scale=0.125,
                                         bias=bias_sb[:, 2:3])
            # ---- votes^2 -> n2 (pair-packed [128,1024]) -> rsq2 ----
            n2ps = [ps5.tile([128, 512], f32, tag="ps5", name=f"n2ps{nh}")
                    for nh in (0, 1)]
            for par in range(2):
                for j in range(8):
                    sq = sqp.tile([128, 1024], bf16, tag="sq")
                    nc.vector.tensor_mul(sq, votes[par][:, j * 1024:(j + 1) * 1024],
                                         votes[par][:, j * 1024:(j + 1) * 1024])
                    for nh in (0, 1):
                        nc.tensor.matmul(
                            n2ps[nh][64 * par:64 * par + 64, :],
                            ored_sb[:, 64 * j:64 * j + 64],
                            sq[:, nh * 512:nh * 512 + 512],
                            start=(j == 0), stop=(j == 7))
            rsq2 = smallp.tile([128, 1024], bf16, tag="rsq2", bufs=2)
            for nh in (0, 1):
                lnn2 = smallp.tile([128, 512], f32, tag="t1", bufs=1)
                nc.scalar.activation(lnn2, n2ps[nh], AF.Ln)
                nc.scalar.activation(rsq2[:, nh * 512:nh * 512 + 512], lnn2,
                                     AF.Exp, scale=-0.5)
            return {"q": q, "pair": pair, "votes": votes, "pa": pa, "rsq2": rsq2}

        def phase_b(st):
            q, pair, votes, pa, rsq2 = (st["q"], st["pair"], st["votes"],
                                        st["pa"], st["rsq2"])
            logits = smallp.tile([128, 1024], f32, tag="logits", bufs=1)
            for it in (1, 2):
                # ---- pr = votes * preact ; dot-reduce (pair-packed) ----
                dps = [ps5.tile([128, 512], f32, tag="ps5", name=f"dps{nh}")
                       for nh in (0, 1)]
                for par in range(2):
                    for j in range(8):
                        pr = prp.tile([128, 1024], bf16, tag="pr")
                        nc.vector.tensor_mul(pr,
                                             votes[par][:, j * 1024:(j + 1) * 1024],
                                             pa[par])
                        for nh in (0, 1):
                            nc.tensor.matmul(
                                dps[nh][64 * par:64 * par + 64, :],
                                ored_sb[:, 64 * j:64 * j + 64],
                                pr[:, nh * 512:nh * 512 + 512],
                                start=(j == 0), stop=(j == 7))
                # preact^2, n1 reduce (both parities into [16,512])
                psq = [None, None]
                for par in range(2):
                    psq[par] = smallp.tile([128, 1024], bf16, tag=f"psq{par}",
                                           bufs=1, name=f"psq{par}")
                    nc.vector.tensor_mul(psq[par], pa[par], pa[par])
                rsq1 = smallp.tile([16, 1024], bf16, tag="rsq1", bufs=2)
                for nh in (0, 1):
                    n1ps = psS.tile([16, 512], f32, tag="sm")
                    nc.tensor.matmul(n1ps, o1e_sb, psq[0][:, nh * 512:nh * 512 + 512],
                                     start=True, stop=False)
                    nc.tensor.matmul(n1ps, o1o_sb, psq[1][:, nh * 512:nh * 512 + 512],
                                     start=False, stop=True)
                    lnn1 = smallp.tile([16, 512], f32, tag="t2", bufs=2)
                    nc.scalar.activation(lnn1, n1ps, AF.Ln)
                    nc.scalar.activation(rsq1[:, nh * 512:nh * 512 + 512], lnn1,
                                         AF.Exp, scale=-0.5)
                    r1e = ps5.tile([128, 512], f32, tag="ps5")
                    nc.tensor.matmul(r1e, e2p_sb, rsq1[:, nh * 512:nh * 512 + 512],
                                     start=True, stop=True)
                    # cos = dps * rsq2 * rsq1e ; logits update
                    c1 = smallp.tile([128, 512], f32, tag="t3", bufs=1)
                    nc.vector.tensor_mul(c1, dps[nh], rsq2[:, nh * 512:nh * 512 + 512])
                    if it == 1:
                        nc.vector.tensor_mul(logits[:, nh * 512:nh * 512 + 512],
                                             c1, r1e)
                    else:
                        c2 = smallp.tile([128, 512], f32, tag="t4", bufs=1)
                        nc.vector.tensor_mul(c2, c1, r1e)
                        nc.vector.tensor_add(logits[:, nh * 512:nh * 512 + 512],
                                             logits[:, nh * 512:nh * 512 + 512], c2)
                # ---- softmax over od: route = exp(logits - lnZ_j) ----
                route = smallp.tile([128, 1024], bf16, tag="route", bufs=1)
                for nh in (0, 1):
                    el = smallp.tile([128, 512], bf16, tag="t5", bufs=1)
                    nc.scalar.activation(el, logits[:, nh * 512:nh * 512 + 512], AF.Exp)
                    zps = psS.tile([16, 512], f32, tag="sm")
                    nc.tensor.matmul(zps, osm2_sb, el, start=True, stop=True)
                    lnz = smallp.tile([16, 512], f32r, tag="t6", bufs=1)
                    nc.scalar.activation(lnz, zps, AF.Ln)
                    lze = ps5.tile([128, 512], f32, tag="ps5")
                    nc.tensor.matmul(lze, ez2_sb, lnz, start=True, stop=True)
                    lm = smallp.tile([128, 512], f32, tag="t7", bufs=1)
                    nc.vector.tensor_sub(lm, logits[:, nh * 512:nh * 512 + 512], lze)
                    nc.scalar.activation(route[:, nh * 512:nh * 512 + 512], lm, AF.Exp)
                # ---- preact = sum_j votes * route_exp + rb (PE j-sum) ----
                for par in range(2):
                    prods = [None] * 8
                    for j in range(8):
                        reps = psR.tile([128, 1024], f32, tag="reps")
                        for nh in (0, 1):
                            nc.tensor.matmul(
                                reps[:, nh * 512:nh * 512 + 512],
                                ebig2_sb[64 * par:64 * par + 64, j * 128:(j + 1) * 128],
                                route[64 * par:64 * par + 64, nh * 512:nh * 512 + 512],
                                start=True, stop=True)
                        prods[j] = prodp.tile([128, 1024], bf16, tag="prods",
                                              name=f"prods{j}")
                        nc.vector.tensor_mul(prods[j],
                                             votes[par][:, j * 1024:(j + 1) * 1024],
                                             reps)
                    pa[par] = pap.tile([128, 1024], bf16, tag=f"pa{par}", bufs=3,
                                       name=f"pan{it}_{q}_{par}")
                    for nh in (0, 1):
                        pps = ps5.tile([128, 512], f32, tag="ps5")
                        for j in range(8):
                            nc.tensor.matmul(
                                pps, ident2_sb,
                                prods[j][:, nh * 512:nh * 512 + 512],
                                start=(j == 0), stop=(j == 7))
                        nc.scalar.activation(pa[par][:, nh * 512:nh * 512 + 512], pps,
                                             AF.Identity, bias=bias_sb[:, 1:2])
            # ---- squash: out = preact * exp(0.5 ln(nsq) - ln(1+nsq)) ----
            psq2 = [None, None]
            for par in range(2):
                psq2[par] = smallp.tile([128, 1024], bf16, tag=f"psq{par}",
                                        bufs=1, name=f"psqs{par}")
                nc.vector.tensor_mul(psq2[par], pa[par], pa[par])
            for nh in (0, 1):
                yps = psS.tile([16, 512], f32, tag="sm")
                nc.tensor.matmul(yps, o1e_sb, psq2[0][:, nh * 512:nh * 512 + 512],
                                 start=True, stop=False)
                nc.tensor.matmul(yps, o1o_sb, psq2[1][:, nh * 512:nh * 512 + 512],
                                 start=False, stop=True)
                u1 = smallp.tile([16, 512], f32, tag="t8", bufs=2)
                nc.scalar.activation(u1, yps, AF.Ln)
                u2 = smallp.tile([16, 512], f32, tag="t9", bufs=2)
                nc.scalar.activation(u2, yps, AF.Ln, bias=1.0)
                farg = smallp.tile([16, 512], f32, tag="t10", bufs=2)
                nc.vector.scalar_tensor_tensor(farg, u1, 0.5, u2,
                                               op0=ALU.mult, op1=ALU.subtract)
                f8 = smallp.tile([16, 512], f32r, tag="t11", bufs=2)
                nc.scalar.activation(f8, farg, AF.Exp)
                for par, p in enumerate(pair):
                    fps = ps5.tile([128, 512], f32, tag="ps5")
                    nc.tensor.matmul(fps, fbig_e_sb if par == 0 else fbig_o_sb,
                                     f8, start=True, stop=True)
                    outt = outp.tile([128, 512], f32, tag="out")
                    nc.vector.tensor_mul(outt, pa[par][:, nh * 512:nh * 512 + 512], fps)
                    nc.sync.dma_start(
                        out_d[:, p * 1024 + nh * 512:p * 1024 + nh * 512 + 512], outt)

        for q in range(4):
            phase_b(phase_a(q))

    nc.compile()
    return nc


# ---------------- public entry point ----------------

def kernel(x, w, deconv_b, routing_bias):
    from concourse.bass_utils import run_bass_kernel_spmd

    x = np.asarray(x, np.float32)
    w = np.asarray(w, np.float32)
    deconv_b = np.asarray(deconv_b, np.float32)
    routing_bias = np.asarray(routing_bias, np.float32)

    if "nc" not in _CACHE:
        _CACHE["nc"] = _build_nc()
    nc = _CACHE["nc"]

    consts = _host_constants(w, deconv_b, routing_bias)
    in_maps = []
    for c in range(8):
        b, s = c // 4, c % 4
        m = dict(consts)
        m["xrep"] = _make_xrep(x, b, s).astype(BF16)
        in_maps.append(m)

    res = run_bass_kernel_spmd(nc, in_maps, list(range(8)),
                               trace=bool(_CACHE.get("trace")))
    _CACHE["last_res"] = res

    out = np.zeros((B, OUT_DIM, OUT_ATOMS, DO, DO, DO), np.float32)
    for c in range(8):
        b, s = c // 4, c % 4
        blk = np.asarray(res.results[c]["out"], np.float32)
        blk = blk.reshape(OUT_DIM, OUT_ATOMS, 2, 2, 2, 4, 16, 16)
        t = blk.transpose(0, 1, 5, 2, 6, 3, 7, 4)  # od,oa,a',rd,bh,rh,bw,rw
        out[b, :, :, 8 * s:8 * s + 8, :, :] = t.reshape(OUT_DIM, OUT_ATOMS, 8, 32, 32)
    return out
